# revision 1
# baseline (speedup 1.0000x reference)
"""Trainium2 Bass kernel for the AttnEncoder LSTM problem.

Reference computation (per timestep t, PyTorch LSTM cell gate order i,f,g,o):
    z1 = relu([h, c] @ W1.T + b1)          # [B, 512]
    z2 = relu(v_t @ W2.T + b2)             # [B, 512]  (recurrence-independent)
    x  = relu([z1, z2] @ W3.T + b3)        # [B, 512]
    gates = x @ Wih.T + bih + h @ Whh.T + bhh
    c' = sig(f)*c + sig(i)*tanh(g);  h' = sig(o)*tanh(c')
Output: h stacked over t -> [B, T, 512].

Strategy: 8-way data parallel over batch (B=1024 -> 128 rows/core, exactly one
SBUF partition tile). Everything on-device is kept feature-major ([feat, batch])
so activations feed the next matmul as the moving operand with no transposes.
Matmul inputs are bf16 (1 cyc/row on PE vs 4 for fp32); all elementwise state
math is fp32. z2 for all timesteps is precomputed into a DRAM scratch first.
"""

import numpy as np
import ml_dtypes

import concourse.bass as bass
import concourse.mybir as mybir
import concourse.tile as tile
from concourse import bacc
from concourse.bass_utils import run_bass_kernel_spmd

F32 = mybir.dt.float32
BF16 = mybir.dt.bfloat16
AF = mybir.ActivationFunctionType
ts = bass.ts

B, T, DP = 1024, 128, 10
H = 512
NCORES = 8
BL = B // NCORES  # 128 batch rows per core

_CACHE = {}
LAST_RESULTS = None


def build(t_steps=T, do_compile=True, repeat=1):
    nc = bacc.Bacc("TRN2", num_devices=NCORES)

    # Pre-transposed weight chunk layouts (built on host):
    #   w1t[p, (k*4+m)*128+q] = W1[128m+q, 128k+p]      k: [h;c] chunks, m: out chunks
    #   w3t[p, (k*4+m)*128+q] = W3[128m+q, 128k+p]      k: [z1;z2] chunks
    #   wgt[p, (k*16+m)*128+q] = [Wih|Whh][128m+q, 128k+p]
    w1t = nc.dram_tensor("w1t", [128, 32 * 128], BF16, kind="ExternalInput")
    w3t = nc.dram_tensor("w3t", [128, 32 * 128], BF16, kind="ExternalInput")
    wgt = nc.dram_tensor("wgt", [128, 128 * 128], BF16, kind="ExternalInput")
    w2t = nc.dram_tensor("w2t", [DP, 512], BF16, kind="ExternalInput")
    svt = nc.dram_tensor("svt", [DP, T * BL], BF16, kind="ExternalInput")
    b1t = nc.dram_tensor("b1t", [128, 4], F32, kind="ExternalInput")
    b3t = nc.dram_tensor("b3t", [128, 4], F32, kind="ExternalInput")
    bgt = nc.dram_tensor("bgt", [128, 16], F32, kind="ExternalInput")
    b2t = nc.dram_tensor("b2t", [128, 4], F32, kind="ExternalInput")
    # out[t, p, 128m+b] = h_t[feature 128m+p, batch b]
    out = nc.dram_tensor("out", [T, 128, 512], F32, kind="ExternalOutput")
    # z2 scratch: z2d[t, m, p, b] = z2_t[feature 128m+p, batch b] (bf16)
    z2d = nc.dram_tensor("z2d", [T, 4, 128, BL], BF16, kind="Internal")

    with tile.TileContext(nc) as tc:
        with (
            tc.tile_pool(name="weights", bufs=1) as wpool,
            tc.tile_pool(name="state", bufs=2) as spool,
            tc.tile_pool(name="work", bufs=2) as wkpool,
            tc.tile_pool(name="z2in", bufs=3) as z2pool,
            tc.tile_pool(name="psum", bufs=1, space="PSUM") as pp,
        ):
            w1 = wpool.tile([128, 32 * 128], BF16)
            nc.sync.dma_start(w1[:], w1t[:, :])
            w3 = wpool.tile([128, 32 * 128], BF16)
            nc.sync.dma_start(w3[:], w3t[:, :])
            wg = wpool.tile([128, 128 * 128], BF16)
            nc.sync.dma_start(wg[:], wgt[:, :])
            b1s = wpool.tile([128, 4], F32)
            nc.sync.dma_start(b1s[:], b1t[:, :])
            b3s = wpool.tile([128, 4], F32)
            nc.sync.dma_start(b3s[:], b3t[:, :])
            bgs = wpool.tile([128, 16], F32)
            nc.sync.dma_start(bgs[:], bgt[:, :])
            b2s = wpool.tile([128, 4], F32)
            nc.sync.dma_start(b2s[:], b2t[:, :])

            # ---------------- phase 1: z2 precompute ----------------
            # z2 = relu(W2 @ v + b2) for all timesteps, staged to a DRAM
            # scratch. Only the first 4 t-groups run upfront; the remaining
            # groups are interleaved into the early recurrence steps (see
            # z2_group below) where their matmuls fill PE stall gaps.
            w2 = wpool.tile([DP, 512], BF16)
            nc.sync.dma_start(w2[:], w2t[:, :])
            sv = wpool.tile([DP, T * BL], BF16)
            nc.sync.dma_start(sv[:], svt[:, :])

            def z2_group(g):
                for m in range(4):
                    ps = pp.tile([128, 512], F32, tag="zps", bufs=1, name="zps")
                    nc.tensor.matmul(
                        ps[:], w2[:, ts(m, 128)], sv[:, ts(g, 512)],
                        start=True, stop=True,
                    )
                    zs = wkpool.tile([128, 512], BF16, tag="zs", bufs=4, name="zs")
                    # relu(ps + b2) with bf16 cast; alternate ACT/DVE so
                    # neither engine serializes this phase.
                    if (g * 4 + m) % 2 == 0:
                        nc.scalar.activation(
                            zs[:], ps[:], AF.Relu, bias=b2s[:, m : m + 1]
                        )
                    else:
                        nc.vector.tensor_scalar(
                            zs[:], ps[:], b2s[:, m : m + 1], 0.0,
                            mybir.AluOpType.add, mybir.AluOpType.max,
                        )
                    nc.sync.dma_start(
                        z2d[4 * g : 4 * g + 4, m].rearrange("t p b -> p t b"),
                        zs[:].rearrange("p (t b) -> p t b", t=4),
                    )

            n_groups = T * BL // 512  # 32 groups of 4 timesteps
            for g in range(min(4, n_groups)):
                z2_group(g)

            # ---------------- phase 2: recurrence over T ----------------
            h_bf = spool.tile([128, 512], BF16, tag="hbf", bufs=2)
            nc.vector.memset(h_bf[:], 0.0)
            c_bf = spool.tile([128, 512], BF16, tag="cbf", bufs=2)
            nc.vector.memset(c_bf[:], 0.0)
            c32 = spool.tile([128, 512], F32, tag="c32", bufs=2)
            nc.vector.memset(c32[:], 0.0)

            funcs = [AF.Sigmoid, AF.Sigmoid, AF.Tanh, AF.Sigmoid]

            # Gate issue order i, g, f, o: the c' chain needs i*g and f*c
            # before tanh; o is only needed for the final h product.
            gorder = [0, 2, 1, 3]

            for rep in range(repeat):
              for t in range(t_steps):
                # interleave one remaining z2 precompute group per early step
                # (8 steps of lead time before its data is consumed)
                if (rep == 0 and t_steps == T and t % 4 == 2
                        and 4 + (t - 2) // 4 < n_groups):
                    z2_group(4 + (t - 2) // 4)

                z2t = z2pool.tile([128, 512], BF16, tag="z2t", bufs=3)
                nc.sync.dma_start(
                    z2t[:].rearrange("p (m b) -> p m b", m=4),
                    z2d[t].rearrange("m p b -> p m b"),
                )

                # One PSUM accumulation group per bank per step: start=True on
                # the bank's first matmul zeroes the whole 2KB bank; stop=True
                # on the bank's last matmul closes the group.

                # x-stage z2 contributions first: they depend only on the z2
                # prefetch, so the PE can run them during the previous step's
                # elementwise tail.
                xps = pp.tile([128, 512], F32, tag="xps", bufs=2)
                for m in range(4):
                    for kz in range(4):
                        k = 4 + kz  # z2 chunk
                        nc.tensor.matmul(
                            xps[:, ts(m, 128)], w3[:, ts(k * 4 + m, 128)],
                            z2t[:, ts(kz, 128)],
                            start=(m == 0 and kz == 0), stop=False,
                        )

                # z1 = relu(W1 @ [h; c] + b1), feature-major. c chunks first
                # (c_bf quarters are ready before h_bf in the previous tail),
                # k-outer so chunks are consumed as they arrive.
                z1ps = pp.tile([128, 512], F32, tag="z1ps", bufs=1)
                for k in [4, 5, 6, 7, 0, 1, 2, 3]:
                    rhs = h_bf[:, ts(k, 128)] if k < 4 else c_bf[:, ts(k - 4, 128)]
                    for m in range(4):
                        nc.tensor.matmul(
                            z1ps[:, ts(m, 128)], w1[:, ts(k * 4 + m, 128)], rhs,
                            start=(m == 0 and k == 4), stop=(m == 3 and k == 3),
                        )

                # gates pass 1: Whh @ h contributions (independent of z1/x) —
                # keeps PE busy while z1/x activations run. Last h chunk is
                # deferred until after the x@z1 matmuls to cover x's relu.
                gps = [
                    pp.tile([128, 512], F32, tag=f"g{i}ps", bufs=1, name=f"g{i}ps")
                    for i in range(4)
                ]

                def gates_mms(k, rhs_tile, kc, start_k, stop_k):
                    for gi in gorder:
                        for j in range(4):
                            mm = gi * 4 + j
                            nc.tensor.matmul(
                                gps[gi][:, ts(j, 128)],
                                wg[:, ts(k * 16 + mm, 128)],
                                rhs_tile[:, ts(kc, 128)],
                                start=(j == 0 and k == start_k),
                                stop=(j == 3 and k == stop_k),
                            )

                for k in range(4, 7):
                    gates_mms(k, h_bf, k - 4, 4, None)

                # relu+bias on DVE (tensor_scalar add/max) — ACT is the busier
                # engine with the gate sigmoids/tanh.
                z1bf = wkpool.tile([128, 512], BF16, tag="z1bf", bufs=2)
                for m in range(4):
                    nc.vector.tensor_scalar(
                        z1bf[:, ts(m, 128)], z1ps[:, ts(m, 128)],
                        b1s[:, m : m + 1], 0.0,
                        mybir.AluOpType.add, mybir.AluOpType.max,
                    )

                # x-stage z1 contributions, k-outer
                for k in range(4):
                    for m in range(4):
                        nc.tensor.matmul(
                            xps[:, ts(m, 128)], w3[:, ts(k * 4 + m, 128)],
                            z1bf[:, ts(k, 128)],
                            start=False, stop=(m == 3 and k == 3),
                        )

                # deferred last gates@h chunk covers the x relu latency
                gates_mms(7, h_bf, 3, 4, None)

                xbf = wkpool.tile([128, 512], BF16, tag="xbf", bufs=2)
                for m in range(4):
                    nc.vector.tensor_scalar(
                        xbf[:, ts(m, 128)], xps[:, ts(m, 128)],
                        b3s[:, m : m + 1], 0.0,
                        mybir.AluOpType.add, mybir.AluOpType.max,
                    )

                # gates pass 2: Wih @ x contributions. Bank-outer with o last:
                # banks i/g/f finish early so their activations and the
                # c' = f*c + i*g chain overlap the remaining pass-2 matmuls.
                for gi in gorder:
                    for k in range(4):
                        for j in range(4):
                            mm = gi * 4 + j
                            nc.tensor.matmul(
                                gps[gi][:, ts(j, 128)],
                                wg[:, ts(k * 16 + mm, 128)],
                                xbf[:, ts(k, 128)],
                                start=False, stop=(k == 3 and j == 3),
                            )

                gsb = [
                    wkpool.tile([128, 512], F32, tag=f"g{i}sb", bufs=2, name=f"g{i}sb")
                    for i in range(4)
                ]
                i_s, f_s, g_s, o_s = gsb

                # Tail in column quarters: gate activations (ACT) feed the
                # c'/h' chain (DVE); c_bf/h_bf quarters are produced directly
                # (bf16) so next-step matmuls unblock as early as possible.
                c32_new = spool.tile([128, 512], F32, tag="c32", bufs=2)
                c_bf_new = spool.tile([128, 512], BF16, tag="cbf", bufs=2)
                h_bf_new = spool.tile([128, 512], BF16, tag="hbf", bufs=2)
                t1 = wkpool.tile([128, 512], F32, tag="t1", bufs=2)
                t2 = wkpool.tile([128, 512], F32, tag="t2", bufs=2)
                th = wkpool.tile([128, 512], F32, tag="th", bufs=2)
                h32 = wkpool.tile([128, 512], F32, tag="h32", bufs=2)
                # Issue quarter q's tanh after quarter q+1's gate activations:
                # the tanh waits on the DVE c' chain, and stalling ACT there
                # would delay the next quarter's sigmoids.
                def tail_tanh(q):
                    qs = ts(q, 128)
                    nc.scalar.activation(th[:, qs], c32_new[:, qs], AF.Tanh)
                    nc.vector.tensor_mul(h_bf_new[:, qs], o_s[:, qs], th[:, qs])
                    nc.vector.tensor_mul(h32[:, qs], o_s[:, qs], th[:, qs])

                for q in range(4):
                    qs = ts(q, 128)
                    for gi in gorder:
                        mm = gi * 4 + q
                        nc.scalar.activation(
                            gsb[gi][:, qs], gps[gi][:, qs],
                            funcs[gi], bias=bgs[:, mm : mm + 1],
                        )
                    nc.vector.tensor_mul(t1[:, qs], i_s[:, qs], g_s[:, qs])
                    nc.vector.tensor_mul(t2[:, qs], f_s[:, qs], c32[:, qs])
                    nc.vector.tensor_add(c32_new[:, qs], t1[:, qs], t2[:, qs])
                    nc.vector.tensor_add(c_bf_new[:, qs], t1[:, qs], t2[:, qs])
                    if q > 0:
                        tail_tanh(q - 1)
                tail_tanh(3)
                c32, c_bf, h_bf = c32_new, c_bf_new, h_bf_new

                nc.sync.dma_start(out[t], h32[:])

    if do_compile:
        nc.compile()
    return nc


def _get_nc():
    if "nc" not in _CACHE:
        _CACHE["nc"] = build()
    return _CACHE["nc"]


def _get_runner():
    """Jitted 8-core executor, cached across calls. Device-side zero outputs
    (donated) avoid shipping the output-sized zero buffers from host."""
    if "runner" in _CACHE:
        return _CACHE["runner"]
    import jax
    from jax.sharding import Mesh, PartitionSpec, NamedSharding

    try:
        from jax.experimental.shard_map import shard_map
    except ImportError:
        from jax import shard_map
    from concourse import bass2jax
    from concourse.bass2jax import _bass_exec_p, partition_id_tensor

    nc = _get_nc()
    bass2jax.install_neuronx_cc_hook()

    partition_name = nc.partition_id_tensor.name if nc.partition_id_tensor else None
    in_names, out_names, out_avals, zero_shapes = [], [], [], []
    for alloc in nc.m.functions[0].allocations:
        if not isinstance(alloc, mybir.MemoryLocationSet):
            continue
        name = alloc.memorylocations[0].name
        if alloc.kind == "ExternalInput":
            if name != partition_name:
                in_names.append(name)
        elif alloc.kind == "ExternalOutput":
            out_names.append(name)
            shape = tuple(alloc.tensor_shape)
            dtype = mybir.dt.np(alloc.dtype)
            out_avals.append(jax.core.ShapedArray(shape, dtype))
            zero_shapes.append((shape, dtype))
    n_params = len(in_names)
    n_outs = len(out_avals)
    all_in_names = list(in_names) + list(out_names)
    if partition_name is not None:
        all_in_names.append(partition_name)
    donate = tuple(range(n_params, n_params + n_outs))

    def _body(*args):
        operands = list(args)
        if partition_name is not None:
            operands.append(partition_id_tensor())
        outs = _bass_exec_p.bind(
            *operands,
            out_avals=tuple(out_avals),
            in_names=tuple(all_in_names),
            out_names=tuple(out_names),
            lowering_input_output_aliases=(),
            sim_require_finite=True,
            sim_require_nnan=True,
            nc=nc,
        )
        return tuple(outs)

    devices = jax.devices()[:NCORES]
    mesh = Mesh(np.asarray(devices), ("core",))
    in_specs = (PartitionSpec("core"),) * (n_params + n_outs)
    out_specs = (PartitionSpec("core"),) * n_outs
    sharded = jax.jit(
        shard_map(
            _body, mesh=mesh, in_specs=in_specs, out_specs=out_specs, check_rep=False
        ),
        donate_argnums=donate,
        keep_unused=True,
    )
    sh = NamedSharding(mesh, PartitionSpec("core"))
    import jax.numpy as jnp

    def make_zeros():
        return [
            jax.jit(
                lambda s=s, d=d: jnp.zeros((NCORES * s[0], *s[1:]), d),
                out_shardings=sh,
            )()
            for (s, d) in zero_shapes
        ]

    runner = dict(
        sharded=sharded, sh=sh, in_names=in_names, out_names=out_names,
        out_avals=out_avals, make_zeros=make_zeros, jax=jax,
    )
    _CACHE["runner"] = runner
    return runner


def _run_fast(in_maps):
    import jax

    r = _get_runner()
    concat_in = [
        np.concatenate([np.asarray(m[nm]) for m in in_maps], axis=0)
        for nm in r["in_names"]
    ]
    dev_in = [jax.device_put(a, r["sh"]) for a in concat_in]
    zs = r["make_zeros"]()
    outs = r["sharded"](*dev_in, *zs)
    results = []
    for c in range(NCORES):
        results.append(
            {
                nm: np.asarray(outs[i]).reshape(NCORES, *r["out_avals"][i].shape)[c]
                for i, nm in enumerate(r["out_names"])
            }
        )
    return results


def kernel(stockvec, W1, b1, W2, b2, W3, b3, Wih, Whh, bih, bhh):
    global LAST_RESULTS
    bf = ml_dtypes.bfloat16
    f32 = np.float32
    stockvec = np.asarray(stockvec, f32)
    W1, b1, W2, b2, W3, b3 = (np.asarray(a, f32) for a in (W1, b1, W2, b2, W3, b3))
    Wih, Whh, bih, bhh = (np.asarray(a, f32) for a in (Wih, Whh, bih, bhh))

    w1t_np = np.ascontiguousarray(
        W1.reshape(4, 128, 8, 128).transpose(3, 2, 0, 1)
    ).reshape(128, 4096).astype(bf)
    w3t_np = np.ascontiguousarray(
        W3.reshape(4, 128, 8, 128).transpose(3, 2, 0, 1)
    ).reshape(128, 4096).astype(bf)
    wcat = np.concatenate([Wih, Whh], axis=1)  # [2048, 1024]
    wgt_np = np.ascontiguousarray(
        wcat.reshape(16, 128, 8, 128).transpose(3, 2, 0, 1)
    ).reshape(128, 16384).astype(bf)
    w2t_np = np.ascontiguousarray(W2.T).astype(bf)  # [10, 512]
    b1t_np = np.ascontiguousarray(b1.reshape(4, 128).T)
    b3t_np = np.ascontiguousarray(b3.reshape(4, 128).T)
    bgt_np = np.ascontiguousarray((bih + bhh).reshape(16, 128).T)
    b2t_np = np.ascontiguousarray(b2.reshape(4, 128).T)

    in_maps = []
    for ci in range(NCORES):
        shard = stockvec[ci * BL : (ci + 1) * BL]  # [BL, T, 10]
        svt_np = np.ascontiguousarray(
            shard.transpose(2, 1, 0).reshape(DP, T * BL)
        ).astype(bf)
        in_maps.append(
            dict(
                w1t=w1t_np, w3t=w3t_np, wgt=wgt_np, w2t=w2t_np, svt=svt_np,
                b1t=b1t_np, b3t=b3t_np, bgt=bgt_np, b2t=b2t_np,
            )
        )

    try:
        results = _run_fast(in_maps)
    except Exception:
        nc = _get_nc()
        res = run_bass_kernel_spmd(nc, in_maps, core_ids=list(range(NCORES)))
        LAST_RESULTS = res
        results = res.results

    outs = []
    for ci in range(NCORES):
        o = results[ci]["out"]  # [T, 128, 512]
        o = o.reshape(T, 128, 4, 128).transpose(3, 0, 2, 1).reshape(BL, T, 512)
        outs.append(o)
    return np.ascontiguousarray(np.concatenate(outs, axis=0)).astype(np.float32)

